# revision 6
# baseline (speedup 1.0000x reference)
"""Deformable conv2d + residual add + ReLU on 8 Trainium2 NeuronCores.

Self-contained harness entry: kernel(**inputs) -> np.ndarray.
Sharding: data-parallel over batch N=8 (one image per core); weight/bias
replicated. Each core runs the same Bass/Tile program.

Design (SWDGE-descgen-bound pipeline, bf16 data path):
  Prologue (ordered so PE transpose work overlaps DVE index math):
  A) zero-padded image planes [x, Dy, Dx, Dxy] in bf16 on DVE (x loaded
     contiguously, cast to bf16 first; padding inserted by a strided-write
     copy), PE-transposed to q-major 1KB rows in DRAM (two 128-blocks per
     PSUM eviction). Plane diffs are chunked so PE transposes start early.
  B) gather-index chain on DVE directly in the SWDGE wrapped layout
     [16, (axis,k,slot)] (offsets pre-wrapped on host) -> no transposes or
     scatter DMAs; floor via round-to-nearest + is_gt correction. A second
     small chain in packed [126, 448] layout produces the bilinear weights,
     PE-transposed per 128-position block into per-partition scalars
     (deferred into the first gather's shadow).
  Main loop, per kernel-tap k:
  C) SWDGE dma_gather of 3200 sample rows (position-major; few large calls
     to amortize the per-call fixed cost), bilinear combine with 2 fused
     scalar_tensor_tensor ops per block ([x|Dy] + wx*[Dx|Dxy], then
     + wy*hi), PE transpose back to channel-major, bf16 matmul accumulated
     in PSUM (3-deep gather buffering to keep descgen back-to-back).
  D) epilogue: x2 added in PSUM via identity matmul (bf16), then per-512-col
     chunk: ReLU+bias on ACT, store f32.

Math: bilinear(x, py, px) = x[q] + wx*Dx[q] + wy*Dy[q] + wx*wy*Dxy[q] with
q = floor(py+PD)*WP + floor(px+PD) on the zero-padded grid; the zero
padding reproduces torchvision's out-of-bounds zeroing exactly, and clamping
floor() into the pad ring keeps fully-out-of-range samples at zero.
The clamp bounds q <= 58*60+58 = 3538, so the table only needs 3584 rows
(28 blocks of 128).
"""

import sys

for _p in ("/opt/trn_rl_repo",):
    if _p not in sys.path:
        sys.path.insert(0, _p)

import numpy as np
import ml_dtypes

import concourse.bacc as bacc
import concourse.mybir as mybir
import concourse.tile as tile
from concourse import bass_utils
from concourse.masks import make_identity

F32 = mybir.dt.float32
BF16 = mybir.dt.bfloat16
I32 = mybir.dt.int32
I16 = mybir.dt.int16
A = mybir.AluOpType
ACTF = mybir.ActivationFunctionType

# problem constants (nn_DeformConvAddReLU2d: N=8, C=Cout=128, 56x56, 3x3)
N, C, H, W = 8, 128, 56, 56
K = 9
PD = 2
HP, WP = H + 2 * PD, W + 2 * PD          # 60, 60
Q = HP * WP                               # 3600
QT = 3584                                 # table rows (28 blocks; q <= 3538)
TPAIR = QT // 256                         # 14 transpose pairs
NPOS = H * W                              # 3136
NPB = 3200                                # samples per tap padded to 25 blocks
NBLK = NPB // 128                         # 25
ELEM = 512                                # row: [x|Dy|Dx|Dxy] x 128c bf16 (1KB)
SLOTS = NPB // 16                         # 200 wrapped idx slots per tap
SPT = 7                                   # 3136 = 7 * 448 partition packing
FREE1 = NPOS // SPT                       # 448
# gather call split (start_block, nblocks); calls >1024 idxs deadlock the
# SWDGE ring (single_packet doorbells only fire at end-of-call).
GSPLIT = [(0, 8), (8, 8), (16, 8), (24, 1)]
# last tap: same split; the trailing 1-block call keeps the tail combine short
GSPLIT_LAST = [(0, 8), (8, 8), (16, 8), (24, 1)]


def host_consts():
    """Base sampling positions, pre-biased by +PD (padded-grid coords).

    Returns:
      based: [126, 448] f32 — deinterleaved packed layout (axis, k, s) x f
             for the weight chain.
      basew: [16, 2*K*SLOTS] f32 — SWDGE-wrapped layout p x (axis, k, slot)
             for the gather-index chain; padded tail positions get -1000 so
             they clamp to q=0 (a guaranteed-zero pad row).
    """
    ki = np.arange(3).repeat(3)
    kj = np.tile(np.arange(3), 3)
    i = np.arange(H)
    j = np.arange(W)
    by = (i[None, :, None] + ki[:, None, None] + 1).astype(np.float32)
    bx = (j[None, None, :] + kj[:, None, None] + 1).astype(np.float32)
    by = np.broadcast_to(by, (K, H, W)).reshape(K, NPOS)
    bx = np.broadcast_to(bx, (K, H, W)).reshape(K, NPOS)
    based = np.concatenate(
        [by.reshape(K * SPT, FREE1), bx.reshape(K * SPT, FREE1)], axis=0
    ).astype(np.float32)

    byp = np.full((K, NPB), -1000.0, dtype=np.float32)
    bxp = np.full((K, NPB), -1000.0, dtype=np.float32)
    byp[:, :NPOS] = by
    bxp[:, :NPOS] = bx
    # wrap: [K, SLOTS, 16] -> [16, K, SLOTS]
    byw = byp.reshape(K, SLOTS, 16).transpose(2, 0, 1).reshape(16, K * SLOTS)
    bxw = bxp.reshape(K, SLOTS, 16).transpose(2, 0, 1).reshape(16, K * SLOTS)
    basew = np.concatenate([byw, bxw], axis=1)
    return based, np.ascontiguousarray(basew)


def wrap_offsets(off):
    """off [2K, NPOS] f32 -> SWDGE-wrapped [16, 2*K*SLOTS] (axis, k, slot)."""
    offp = np.zeros((2 * K, NPB), dtype=np.float32)
    offp[:, :NPOS] = off
    w = offp.reshape(K, 2, SLOTS, 16).transpose(3, 1, 0, 2)  # [16, 2, K, SLOTS]
    return np.ascontiguousarray(w.reshape(16, 2 * K * SLOTS))


def build_kernel(tc, outs, ins):
    nc = tc.nc
    out_d = outs                                   # [128, NPOS] f32
    x_d, offd_d, offw_d, x2_d, wt_d, bias_d, based_d, basew_d = ins

    with tc.tile_pool(name="persist", bufs=1) as pers, \
         tc.tile_pool(name="dram", bufs=1, space="DRAM") as dp:
        g4r = dp.tile([QT, ELEM], BF16)
        wd = dp.tile([126, FREE1], F32)

        idn = pers.tile([128, 128], F32)
        make_identity(nc, idn[:])
        idnb = pers.tile([128, 128], BF16)
        nc.vector.tensor_copy(out=idnb[:], in_=idn[:])
        wsc = pers.tile([128, NBLK, 18], BF16)     # scalars: wy at k, wx at 9+k
        nc.vector.memset(wsc[:], 0.0)
        idxw = pers.tile([128, K * SLOTS], I16)    # wrapped gather indices
        w_sb = pers.tile([128, K * 128], BF16)     # lhsT per tap: [c, o]
        bias_sb = pers.tile([128, 1], F32)
        x2b = pers.tile([128, NPOS], BF16)

        # =============== Prologue ==================
        with tc.tile_pool(name="prosb", bufs=1) as sp, \
             tc.tile_pool(name="proev", bufs=3) as evp, \
             tc.tile_pool(name="props", bufs=3, space="PSUM") as pp, \
             tc.tile_pool(name="props2", bufs=2, space="PSUM") as pp2:
            # small loads that gate the longest dependency chains go first
            wa = sp.tile([16, 2 * K * SLOTS], F32, tag="wa")
            wb = sp.tile([16, 2 * K * SLOTS], F32, tag="wb")
            nc.sync.dma_start(out=wa[:], in_=offw_d[:])    # dv (wrapped)
            nc.sync.dma_start(out=wb[:], in_=basew_d[:])   # base (wrapped)
            xf = sp.tile([128, NPOS], F32, tag="xf")
            nc.sync.dma_start(out=xf[:], in_=x_d[:])
            nc.sync.dma_start(out=w_sb[:], in_=wt_d[:])
            nc.sync.dma_start(out=x2b[:], in_=x2_d[:])
            nc.sync.dma_start(out=bias_sb[:], in_=bias_d[:])

            # ---- Phase 1a: gather indices, wrapped layout, on DVE ----
            # 4 rotating [16, M] buffers (wa/wb/wc f32, wi i32) keep SBUF flat
            M = 2 * K * SLOTS                      # 3600
            wc = sp.tile([16, M], F32, tag="wc")
            wi = sp.tile([16, M], I32, tag="wi")
            nc.vector.tensor_tensor(out=wc[:], in0=wa[:], in1=wb[:], op=A.add)
            nc.vector.tensor_scalar(out=wa[:], in0=wc[:], scalar1=0.0,
                                    scalar2=58.0, op0=A.max, op1=A.min)  # tcl
            nc.vector.tensor_copy(out=wi[:], in_=wa[:])    # round-to-nearest
            nc.vector.tensor_copy(out=wb[:], in_=wi[:])    # rf
            nc.vector.tensor_tensor(out=wc[:], in0=wb[:], in1=wa[:],
                                    op=A.is_gt)            # rf > tcl
            nc.vector.tensor_tensor(out=wa[:], in0=wb[:], in1=wc[:],
                                    op=A.subtract)         # floor
            qfw = sp.tile([16, K * SLOTS], F32, tag="qfw")
            nc.vector.scalar_tensor_tensor(
                out=qfw[:], in0=wa[:, :K * SLOTS], scalar=float(WP),
                in1=wa[:, K * SLOTS:], op0=A.mult, op1=A.add)
            nc.vector.tensor_copy(out=idxw[:16, :], in_=qfw[:])
            for r in (16, 32, 64):
                nc.sync.dma_start(out=idxw[r:2 * r, :], in_=idxw[0:r, :])

            # ---- Phase 2: padded planes in bf16, on DVE ----
            xb = sp.tile([128, NPOS], BF16, tag="xb")
            nc.vector.tensor_copy(out=xb[:], in_=xf[:])    # f32 -> bf16
            xpb = sp.tile([128, Q], BF16, tag="xpb")
            nc.vector.memset(xpb[:], 0.0)
            xpv = xpb[:].rearrange("c (h w) -> c h w", h=HP)
            nc.vector.tensor_copy(
                out=xpv[:, PD:PD + H, PD:PD + W],
                in_=xb[:].rearrange("c (h w) -> c h w", h=H))
            dxb = sp.tile([128, Q], BF16, tag="dxb")
            nc.vector.memset(dxb[:, Q - 1:], 0.0)
            nc.vector.tensor_tensor(out=dxb[:, :Q - 1], in0=xpb[:, 1:Q],
                                    in1=xpb[:, :Q - 1], op=A.subtract)
            dyb = sp.tile([128, Q], BF16, tag="dyb")
            nc.vector.memset(dyb[:, Q - WP:], 0.0)
            dxyb = sp.tile([128, Q], BF16, tag="dxyb")
            nc.vector.memset(dxyb[:, Q - WP - 1:], 0.0)

            planes = [xpb, dyb, dxb, dxyb]
            # chunk the remaining diffs + transposes so PE starts early:
            # 7 chunks of 512 q-cols = 2 transpose-pairs each
            for ch in range(7):
                lo = 512 * ch
                hi = min(512 * (ch + 1), QT)
                # dyb/dxyb for this chunk (dxb is complete already)
                hy = min(hi, Q - WP)
                nc.vector.tensor_tensor(out=dyb[:, lo:hy], in0=xpb[:, lo + WP:hy + WP],
                                        in1=xpb[:, lo:hy], op=A.subtract)
                he = min(hi, Q - WP - 1)
                nc.vector.tensor_tensor(out=dxyb[:, lo:he],
                                        in0=dxb[:, lo + WP:he + WP],
                                        in1=dxb[:, lo:he], op=A.subtract)
                for pair in range(2 * ch, 2 * ch + 2):
                    b0 = 2 * pair
                    pt = pp.tile([128, 2 * ELEM], BF16)
                    for bi in range(2):
                        b = b0 + bi
                        for t, pl in enumerate(planes):
                            nc.tensor.transpose(
                                out=pt[:, bi * ELEM + 128 * t:
                                       bi * ELEM + 128 * (t + 1)],
                                in_=pl[:, b * 128:(b + 1) * 128],
                                identity=idnb[:])
                    ev = evp.tile([128, 2 * ELEM], BF16)
                    nc.scalar.copy(out=ev[:], in_=pt[:])
                    nc.sync.dma_start(
                        out=g4r[b0 * 128:(b0 + 2) * 128, :].rearrange(
                            "(b p) e -> p b e", b=2),
                        in_=ev[:].rearrange("p (b e) -> p b e", b=2))

            # ---- Phase 1b: bilinear weights, packed layout, on DVE ----
            # (PE transposes here run in the shadow of the first gather)
            dv = sp.tile([126, FREE1], F32, tag="dv")
            nc.sync.dma_start(out=dv[:], in_=offd_d[:])
            bs = sp.tile([126, FREE1], F32, tag="bs")
            nc.sync.dma_start(out=bs[:], in_=based_d[:])
            tr = sp.tile([126, FREE1], F32, tag="tr")
            nc.vector.tensor_tensor(out=tr[:], in0=dv[:], in1=bs[:], op=A.add)
            tcl = sp.tile([126, FREE1], F32, tag="tcl")
            nc.vector.tensor_scalar(out=tcl[:], in0=tr[:], scalar1=0.0,
                                    scalar2=58.0, op0=A.max, op1=A.min)
            ri = sp.tile([126, FREE1], I32, tag="ri")
            nc.vector.tensor_copy(out=ri[:], in_=tcl[:])
            rf = sp.tile([126, FREE1], F32, tag="rf")
            nc.vector.tensor_copy(out=rf[:], in_=ri[:])
            gtt = sp.tile([126, FREE1], F32, tag="gtt")
            nc.vector.tensor_tensor(out=gtt[:], in0=rf[:], in1=tcl[:],
                                    op=A.is_gt)
            fl = sp.tile([126, FREE1], F32, tag="fl")
            nc.vector.tensor_tensor(out=fl[:], in0=rf[:], in1=gtt[:],
                                    op=A.subtract)
            wv = sp.tile([126, FREE1], F32, tag="wv")    # wy | wx
            nc.vector.tensor_tensor(out=wv[:], in0=tr[:], in1=fl[:],
                                    op=A.subtract)
            # reshuffle [126, 448] (a,k,s)xf -> [18, 3136] (a,k)x(s,f)
            # via DRAM (cross partition/free regrouping needs a flat hop)
            nc.sync.dma_start(out=wd[:], in_=wv[:])
            wsb2 = sp.tile([18, NPOS], F32, tag="wsb2")
            nc.sync.dma_start(
                out=wsb2[:],
                in_=wd[:].rearrange("(c s) f -> c (s f)", s=SPT))
            for b in range(NBLK):
                n = min(128, NPOS - b * 128)
                if n <= 0:
                    break
                ptw = pp2.tile([128, 32], F32)
                nc.tensor.transpose(out=ptw[:n, 0:18],
                                    in_=wsb2[:, b * 128:b * 128 + n],
                                    identity=idn[:18, :18])
                nc.scalar.copy(out=wsc[:n, b, :], in_=ptw[:n, 0:18])

        # ---------------- Phase 3: gather / combine / matmul ----------------
        with tc.tile_pool(name="gk", bufs=3) as gp, \
             tc.tile_pool(name="cp", bufs=2) as cpp, \
             tc.tile_pool(name="cols", bufs=2) as csp, \
             tc.tile_pool(name="uv", bufs=4) as uvp, \
             tc.tile_pool(name="accp", bufs=1, space="PSUM") as accp, \
             tc.tile_pool(name="tps", bufs=1, space="PSUM") as tpp:
            acc = accp.tile([128, NPOS], F32)
            # residual x2 seeds the PSUM accumulation (runs while PE is idle
            # during the first gather's descgen)
            for ch in range(7):
                lo = 512 * ch
                hi = min(lo + 512, NPOS)
                nc.tensor.matmul(acc[:, lo:hi], lhsT=idnb[:],
                                 rhs=x2b[:, lo:hi], start=True, stop=False)
            for k in range(K):
                gk = gp.tile([128, NBLK, ELEM], BF16)
                split = GSPLIT_LAST if k == K - 1 else GSPLIT
                for hb, nb in split:
                    nc.gpsimd.dma_gather(
                        gk[:, hb:hb + nb, :], g4r[:],
                        idxw[:, k * SLOTS + hb * 8:k * SLOTS + (hb + nb) * 8],
                        num_idxs=nb * 128, num_idxs_reg=nb * 128,
                        elem_size=ELEM)
                colsP = cpp.tile([128, NPB], BF16)     # pos-major combined
                for b in range(NBLK):
                    wys = wsc[:, b, k:k + 1]
                    wxs = wsc[:, b, 9 + k:10 + k]
                    uv = uvp.tile([128, 256], BF16, tag="uv")
                    # uv = [x|Dy] + wx*[Dx|Dxy]  ->  [v', u']
                    nc.vector.scalar_tensor_tensor(
                        uv[:], gk[:, b, 256:512], wxs, gk[:, b, 0:256],
                        op0=A.mult, op1=A.add)
                    # cols = v' + wy*u'
                    nc.vector.scalar_tensor_tensor(
                        colsP[:, b * 128:(b + 1) * 128], uv[:, 128:256], wys,
                        uv[:, 0:128], op0=A.mult, op1=A.add)
                cols = csp.tile([128, NPB], BF16)      # c-major
                for g in range(7):
                    bs_ = list(range(4 * g, min(4 * g + 4, NBLK)))
                    ptc = tpp.tile([128, 512], BF16)
                    for j, b in enumerate(bs_):
                        nc.tensor.transpose(out=ptc[:, 128 * j:128 * (j + 1)],
                                            in_=colsP[:, b * 128:(b + 1) * 128],
                                            identity=idnb[:])
                    wdt = len(bs_) * 128
                    nc.scalar.copy(out=cols[:, 512 * g:512 * g + wdt],
                                   in_=ptc[:, :wdt])
                for ch in range(7):
                    lo = 512 * ch
                    hi = min(lo + 512, NPOS)
                    nc.tensor.matmul(acc[:, lo:hi],
                                     lhsT=w_sb[:, k * 128:(k + 1) * 128],
                                     rhs=cols[:, lo:hi],
                                     start=False, stop=(k == K - 1))

            # ------- epilogue: chunked ReLU + store -------
            outp = cpp.tile([128, NPOS], F32, tag="epi2")
            for ch in range(7):
                lo = 512 * ch
                hi = min(lo + 512, NPOS)
                nc.scalar.activation(outp[:, lo:hi], acc[:, lo:hi],
                                     ACTF.Relu, bias=bias_sb[:], scale=1.0)
                nc.sync.dma_start(out=out_d[:, lo:hi], in_=outp[:, lo:hi])


def make_core_inputs(x, offset, weight, bias, x2):
    """Full inputs -> list of 8 per-core input dicts (host batch sharding)."""
    based, basew = host_consts()
    wt = np.ascontiguousarray(
        weight.reshape(128, 128, K).transpose(1, 2, 0).reshape(128, K * 128)
    ).astype(ml_dtypes.bfloat16)
    cores = []
    for i in range(N):
        off = offset[i].reshape(2 * K, NPOS).astype(np.float32)
        offd = np.ascontiguousarray(
            off.reshape(K, 2, SPT, FREE1).transpose(1, 0, 2, 3)
            .reshape(2 * K * SPT, FREE1))
        cores.append({
            "x": np.ascontiguousarray(x[i].reshape(C, NPOS), dtype=np.float32),
            "offd": offd,
            "offw": wrap_offsets(off),
            "x2": np.ascontiguousarray(
                x2[i].reshape(C, NPOS)).astype(ml_dtypes.bfloat16),
            "wt": wt,
            "bias": np.ascontiguousarray(bias.reshape(C, 1), dtype=np.float32),
            "based": based,
            "basew": basew,
        })
    return cores


_CACHED_NC = None

IN_SPECS = [("x", (C, NPOS), F32), ("offd", (2 * K * SPT, FREE1), F32),
            ("offw", (16, 2 * K * SLOTS), F32), ("x2", (C, NPOS), BF16),
            ("wt", (C, K * 128), BF16), ("bias", (C, 1), F32),
            ("based", (2 * K * SPT, FREE1), F32),
            ("basew", (16, 2 * K * SLOTS), F32)]


def _build_nc():
    global _CACHED_NC
    if _CACHED_NC is not None:
        return _CACHED_NC
    nc = bacc.Bacc("TRN2", target_bir_lowering=False, debug=False, num_devices=N)
    ins = [nc.dram_tensor(nm, list(sh), dt, kind="ExternalInput").ap()
           for nm, sh, dt in IN_SPECS]
    out = nc.dram_tensor("out", [C, NPOS], F32, kind="ExternalOutput").ap()
    with tile.TileContext(nc, trace_sim=False) as tc:
        build_kernel(tc, out, ins)
    nc.compile()
    _CACHED_NC = nc
    return nc


def run_cores(inputs, trace=False):
    """Run the SPMD kernel; returns (out [N,C,H,W] f32, exec_time_ns or None)."""
    nc = _build_nc()
    in_maps = make_core_inputs(inputs["x"], inputs["offset"], inputs["weight"],
                               inputs["bias"], inputs["x2"])
    res = bass_utils.run_bass_kernel_spmd(nc, in_maps, core_ids=list(range(N)),
                                          trace=trace)
    out = np.stack([res.results[i]["out"] for i in range(N)])
    return out.reshape(N, C, H, W), res.exec_time_ns


def kernel(x, offset, weight, bias, x2):
    x = np.asarray(x, dtype=np.float32)
    offset = np.asarray(offset, dtype=np.float32)
    weight = np.asarray(weight, dtype=np.float32)
    bias = np.asarray(bias, dtype=np.float32)
    x2 = np.asarray(x2, dtype=np.float32)
    out, _ = run_cores({"x": x, "offset": offset, "weight": weight,
                        "bias": bias, "x2": x2}, trace=False)
    return out
